# revision 16
# baseline (speedup 1.0000x reference)
"""InfoNCE loss on 8 Trainium2 NeuronCores (Bass/Tile, SPMD).

Problem: out [512,128] queries, keys [512,512,128] per-bag banks,
self_index [512]. loss = mean(-lse_pos + log(511) + lse_total) over
logits = einsum('bd,nkd->bnk', out, keys)/0.07 with the self logit
masked by -1e12.

Sharding: keys (bags) are split 8 ways -- each core owns 64 bags
(32768 key columns) and scores ALL 512 queries against them, so every
key byte crosses HBM exactly once (memory roofline).  Queries are
replicated, pre-scaled by 1/T and pre-transposed to [d, q] on the
host; each core's query order is permuted so its own-bag ("diagonal")
queries are local rows 0..63, making the program core-independent.

Per core the device computes, for every query row, per-chunk
(max, sum(exp(l - max))) pairs over its 32768 columns:
  - query group 0 (local rows 0..127) uses per-bag 512-wide chunks;
    the self mask is accumulated into the psum tile by a rank-1
    matmul (onehot_row^T @ mask_row), and the [row p, chunk p]
    diagonal of the stats is exactly the positive-part sum.
  - groups 1..3 use 2048-wide chunks (4 psum banks) to amortize
    vector/scalar instruction overheads.
Max is a DVE reduce (negated, used directly as the exp bias), exp+sum
is one ACT activation with accum_out, written in-place to psum.
The host merges the tiny [128,112] per-core stats in fp64.
"""

import os
import sys

import numpy as np

for _p in (
    "/root/.axon_site",
    "/root/.axon_site/_ro/trn_rl_repo",
    "/root/.axon_site/_ro/pypackages",
    "/opt/trn_rl_repo",
):
    if os.path.isdir(_p) and _p not in sys.path:
        sys.path.append(_p)

import concourse.bass as bass  # noqa: E402
import concourse.tile as tile  # noqa: E402
from concourse import bacc, mybir  # noqa: E402
from concourse.bass_utils import run_bass_kernel_spmd  # noqa: E402

B, K, D = 512, 512, 128
NCORES = 8
BAGS = B // NCORES            # 64 bags per core
LK = BAGS * K                 # 32768 local key columns per core
TEMP = 0.07
NTILE = LK // 512             # 64 bag-aligned key tiles
NSEG = 8
SEG = LK // NSEG              # 4096 keys per DMA segment
G0_COLS = NTILE               # 64 per-bag stat columns for group 0
CHUNK = 2048                  # groups 1..3 chunk width (4 psum banks)
NCH = LK // CHUNK             # 16 chunks per group
NCOLS = G0_COLS + 3 * NCH     # 112 stat columns
ZEROS_CNT = float(B * K - K)  # 261632 label-0 terms contributing exp(0)=1
NUM_P = float(K - 1)          # 511

F32 = mybir.dt.float32
F16 = mybir.dt.float16

_cache: dict = {}


def _build_program():
    nc = bacc.Bacc(
        "TRN2",
        target_bir_lowering=False,
        debug=False,
        enable_asserts=False,
        num_devices=NCORES,
    )
    qT_d = nc.dram_tensor("qT", [D, B], F16, kind="ExternalInput")
    keysT_d = nc.dram_tensor("keysT", [D, LK], F16, kind="ExternalInput")
    negmax_d = nc.dram_tensor("negmax", [128, NCOLS], F32, kind="ExternalOutput")
    sums_d = nc.dram_tensor("sums", [128, NCOLS], F32, kind="ExternalOutput")

    EXP = mybir.ActivationFunctionType.Exp
    AX = mybir.AxisListType.X
    MAX = mybir.AluOpType.max

    with tile.TileContext(nc) as tc:
        from contextlib import ExitStack

        with ExitStack() as ctx:
            consts = ctx.enter_context(tc.tile_pool(name="consts", bufs=1))
            stats = ctx.enter_context(tc.tile_pool(name="stats", bufs=1))
            kpool = ctx.enter_context(tc.tile_pool(name="keys", bufs=1))

            qT = consts.tile([D, B], F16, tag="qT", name="qT_sb")
            negmax_t = stats.tile([128, NCOLS], F32, tag="negmax", name="negmax_sb")
            sums_t = stats.tile([128, NCOLS], F32, tag="sums", name="sums_sb")
            ksegs = [kpool.tile([D, SEG], F16, tag=f"k{s}", name=f"kseg{s}") for s in range(NSEG)]

            nc.sync.dma_start(qT[:], qT_d.ap())
            for s in range(NSEG):
                nc.sync.dma_start(ksegs[s][:], keysT_d.ap()[:, s * SEG:(s + 1) * SEG])

            def rhs_ap(kc):
                s, off = divmod(kc * 512, SEG)
                return ksegs[s][:, off:off + 512]

            qTr = qT[:]

            # Two psum pools alive together (4+4 banks): the ACT-paced
            # group-0 tiles fill PE bubbles of the PE-paced group-1..3
            # chunks and vice versa.
            with tc.tile_pool(name="psum0", bufs=4, space="PSUM") as pp0, \
                 tc.tile_pool(name="psum123", bufs=1, space="PSUM") as pp1:

                def g0_tile(kc):
                    pt = pp0.tile([128, 512], F32, tag="p0", name=f"p0_{kc}")
                    nc.tensor.matmul(
                        pt[:], qTr[:, 0:128], rhs_ap(kc), start=True, stop=True
                    )
                    nc.vector.tensor_reduce(
                        negmax_t[:, kc:kc + 1], pt[:], axis=AX, op=MAX, negate=True
                    )
                    nc.scalar.activation(
                        pt[:],
                        pt[:],
                        EXP,
                        bias=negmax_t[:, kc:kc + 1],
                        scale=1.0,
                        accum_out=sums_t[:, kc:kc + 1],
                    )

                def g123_chunk(g, j):
                    pc = pp1.tile([128, CHUNK], F32, tag="p123", name=f"p123_{g}_{j}")
                    for u in range(4):
                        nc.tensor.matmul(
                            pc[:, u * 512:(u + 1) * 512],
                            qTr[:, g * 128:(g + 1) * 128],
                            rhs_ap(j * 4 + u),
                            start=True,
                            stop=True,
                        )
                    col = G0_COLS + (g - 1) * NCH + j
                    nc.vector.tensor_reduce(
                        negmax_t[:, col:col + 1], pc[:], axis=AX, op=MAX,
                        negate=True,
                    )
                    nc.scalar.activation(
                        pc[:],
                        pc[:],
                        EXP,
                        bias=negmax_t[:, col:col + 1],
                        scale=1.0,
                        accum_out=sums_t[:, col:col + 1],
                    )

                for j in range(NCH):
                    for u in range(4):
                        g0_tile(4 * j + u)
                    for g in range(1, 4):
                        g123_chunk(g, j)

            nc.sync.dma_start(negmax_d.ap(), negmax_t[:])
            nc.sync.dma_start(sums_d.ap(), sums_t[:])

    nc.compile()
    return nc


def get_program():
    if "nc" not in _cache:
        _cache["nc"] = _build_program()
    return _cache["nc"]


def prep_inputs(out, keys, self_index):
    out = np.asarray(out, dtype=np.float32)
    keys = np.asarray(keys, dtype=np.float32)
    si = np.asarray(self_index).astype(np.int64)
    invT = np.float32(1.0 / TEMP)

    in_maps = []
    perms = []
    for c in range(NCORES):
        own = np.arange(c * BAGS, (c + 1) * BAGS)
        rest = np.concatenate(
            [np.arange(0, c * BAGS), np.arange((c + 1) * BAGS, B)]
        )
        perm = np.concatenate([own, rest])  # local row -> global query
        perms.append(perm)
        qT = np.ascontiguousarray((out[perm] * invT).T.astype(np.float16))
        keysT = np.ascontiguousarray(
            keys[c * BAGS:(c + 1) * BAGS].reshape(LK, D).T.astype(np.float16)
        )
        in_maps.append({"qT": qT, "keysT": keysT})
    return in_maps, perms


def host_pos_stats(out, keys, self_index):
    """Masked own-bag stats per row, fp64, from the same fp16 values the
    device consumes.  Returns (m_h, s_h): max and sum(exp(l - max)) over
    the 511 unmasked own-bag logits of each query."""
    out = np.asarray(out, dtype=np.float32)
    keys = np.asarray(keys, dtype=np.float32)
    si = np.asarray(self_index).astype(np.int64)
    q16 = (out * np.float32(1.0 / TEMP)).astype(np.float16).astype(np.float64)
    k16 = keys.astype(np.float16).astype(np.float64)
    l = np.einsum("id,ikd->ik", q16, k16)  # [B, K] own-bag logits
    l[np.arange(B), si] = -np.inf          # exclude self exactly
    m_h = l.max(axis=1)
    s_h = np.exp(l - m_h[:, None]).sum(axis=1)
    return m_h, s_h


def combine(results, perms, m_h, s_h):
    """Merge per-core (negmax, sums) stats into the scalar loss (fp64).

    For each diagonal row, the device's own-bag tile stats (which include
    the unmasked self logit) are replaced by the host fp64 masked stats
    (m_h, s_h) -- both in the total logsumexp and as the positive part."""
    lse_parts = np.empty((NCORES, B))  # per-core partial lse per global row
    dp = np.arange(BAGS)
    for c in range(NCORES):
        m = -results[c]["negmax"].astype(np.float64)  # [128, NCOLS] maxes
        s = results[c]["sums"].astype(np.float64)
        g_rows = perms[c][dp]                          # global ids of diag rows
        m[dp, dp] = m_h[g_rows]
        s[dp, dp] = s_h[g_rows]
        # partial logsumexp over this core's 32768 columns, per local row
        L = np.empty(B)
        for g in range(4):
            cols = (
                slice(0, G0_COLS)
                if g == 0
                else slice(G0_COLS + (g - 1) * NCH, G0_COLS + g * NCH)
            )
            mg = m[:, cols]
            sg = s[:, cols]
            mloc = mg.max(axis=1, keepdims=True)
            L[g * 128:(g + 1) * 128] = (
                mloc[:, 0] + np.log((sg * np.exp(mg - mloc)).sum(axis=1))
            )
        inv = np.argsort(perms[c])
        lse_parts[c] = L[inv]

    lse_total = np.logaddexp.reduce(lse_parts, axis=0)
    pos_log = m_h + np.log(s_h)
    lse_pos = np.logaddexp(np.log(ZEROS_CNT), pos_log)
    per_row = -lse_pos + np.log(NUM_P) + lse_total
    return np.float32(per_row.mean())


def run_device(in_maps, trace=False, **kw):
    nc = get_program()
    return run_bass_kernel_spmd(
        nc, in_maps, core_ids=list(range(NCORES)), trace=trace, **kw
    )


def kernel(out, keys, self_index):
    in_maps, perms = prep_inputs(out, keys, self_index)
    res = run_device(in_maps)
    m_h, s_h = host_pos_stats(out, keys, self_index)
    return combine(res.results, perms, m_h, s_h)
